# revision 27
# baseline (speedup 1.0000x reference)
"""DTCWT 3-level inverse on 8 Trainium2 NeuronCores.

Every filtering stage is a banded matmul on the tensor engine in fp16
(PSUM accumulates fp32; ~7e-4 total rel err vs the 2e-2 gate).

All stages use "data as lhsT" mode: matmul(out, lhsT=data[K=h, M=w],
rhs=mat[K=h, N=h_out]) contracts over the partition dim of the data and
yields the filtered image TRANSPOSED ([w, h_out]); column and row stages
then alternate orientation naturally with zero explicit transposes.

The c2q band construction is folded into the matrices; at L1 the lowpass
path is additionally merged into the band polyphase layout ([E|O] w-planes)
so the final row stage is 4 accumulation passes instead of 6.

Schedule: phase-major (L3 x16, L2 x16, L1 x16) with double/triple-buffered
PSUM pools so neighbouring images' matmuls hide each other's copy latency.
DMA queue slots cost ~600ns regardless of size, so all loads are batched
into a handful of giant multi-dim DMAs (2 matrix blobs, 9 input sweeps,
1 store per image).

Sharding: pure data parallel over batch N (8 cores x 16 channels each).
"""
import sys

for _p in ('/opt/trn_rl_repo',):
    if _p not in sys.path:
        sys.path.append(_p)

import numpy as np
import concourse.bass as bass
import concourse.mybir as mybir
from concourse.tile import TileContext
from concourse.bass_utils import run_bass_kernel_spmd

SQRT_HALF = 0.7071067811865476
N_CORES = 8
IMGS_PER_CORE = 16
F32 = mybir.dt.float32
F16 = mybir.dt.float16


# ---------------------------------------------------------------------------
# Host-side matrix construction (numpy, float64)
# ---------------------------------------------------------------------------
def _conv_rows_valid(x, h):
    hr = h[::-1]
    taps = h.shape[0]
    n = x.shape[-2] - taps + 1
    out = hr[0] * x[..., 0:n, :]
    for k in range(1, taps):
        out = out + hr[k] * x[..., k:k + n, :]
    return out


def _pad_rows_symmetric(x, m):
    pad = [(0, 0)] * (x.ndim - 2) + [(m, m), (0, 0)]
    return np.pad(x, pad, mode='symmetric')


def _colfilter(x, h):
    return _conv_rows_valid(_pad_rows_symmetric(x, h.shape[0] // 2), h)


def _colifilt(x, ha, hb, highpass):
    m = ha.shape[0]
    m2 = m // 2
    r = x.shape[-2]
    xp = _pad_rows_symmetric(x, m2)
    xe = xp[..., 1:r + m - 2:2, :]
    xo = xp[..., 2:r + m - 1:2, :]
    xa, xb = (xe, xo) if highpass else (xo, xe)
    hao, hae = ha[0::2], ha[1::2]
    hbo, hbe = hb[0::2], hb[1::2]
    y0 = _conv_rows_valid(xb, hao)
    y1 = _conv_rows_valid(xa, hbo)
    y2 = _conv_rows_valid(xb, hae)
    y3 = _conv_rows_valid(xa, hbe)
    y = np.stack([y0, y1, y2, y3], axis=-2)
    return y.reshape(y.shape[:-3] + (2 * r, y.shape[-1]))


def _op_matrix(op, n):
    """M[h_in, h_out] with out[h_out, w] = sum_h M[h, h_out] x[h, w]."""
    return np.ascontiguousarray(op(np.eye(n, dtype=np.float64)).T)


def build_matrices(g0o, g1o, g0a, g0b, g1a, g1b):
    """All device matrices as {name: fp16 ndarray}."""
    g0o = np.asarray(g0o, np.float64)
    g1o = np.asarray(g1o, np.float64)
    g0a = np.asarray(g0a, np.float64)
    g0b = np.asarray(g0b, np.float64)
    g1a = np.asarray(g1a, np.float64)
    g1b = np.asarray(g1b, np.float64)
    s = SQRT_HALF
    hs, vs = np.hstack, np.vstack
    out = {}

    def upsample_level(R, tag):
        Mlo = _op_matrix(lambda x: _colifilt(x, g0b, g0a, False), R)  # [R, 2R]
        Mhi = _op_matrix(lambda x: _colifilt(x, g1b, g1a, True), R)
        Me_h, Mo_h = s * Mhi[0::2], s * Mhi[1::2]                     # [R/2, 2R]
        Me_l, Mo_l = s * Mlo[0::2], s * Mlo[1::2]
        out[f'M{tag}_lo'] = Mlo
        # pair-stacked [w1; w2] col rhs, e|o column-concatenated
        #   e: w1r*Me + w2r*Me + w1i*Mo - w2i*Mo
        #   o: -w1r*Mo + w2r*Mo + w1i*Me + w2i*Me
        out[f'L{tag}_hi_R'] = hs([vs([Me_h, Me_h]), vs([-Mo_h, Mo_h])])
        out[f'L{tag}_hi_I'] = hs([vs([Mo_h, -Mo_h]), vs([Me_h, Me_h])])
        out[f'L{tag}_lo_R'] = hs([vs([Me_l, Me_l]), vs([-Mo_l, Mo_l])])
        out[f'L{tag}_lo_I'] = hs([vs([Mo_l, -Mo_l]), vs([Me_l, Me_l])])
        # row stage (polyphase-column recombination)
        out[f'Be{tag}_lo'], out[f'Bo{tag}_lo'] = Mlo[0::2], Mlo[1::2]
        out[f'Be{tag}_hi'], out[f'Bo{tag}_hi'] = Mhi[0::2], Mhi[1::2]

    upsample_level(64, '3')
    upsample_level(128, '2')
    # L3 quad stacks: [hl pair (lo mats); hh pair (hi mats)], K=128
    out['L3_q_R'] = vs([out['L3_lo_R'], out['L3_hi_R']])
    out['L3_q_I'] = vs([out['L3_lo_I'], out['L3_hi_I']])
    del out['L3_lo_R'], out['L3_lo_I']  # only used inside the quad at L3

    # L1 (colfilter, size-preserving, n=256)
    A_lo = _op_matrix(lambda x: _colfilter(x, g0o), 256)              # [256, 256]
    A_hi = _op_matrix(lambda x: _colfilter(x, g1o), 256)
    out['Alo_a'], out['Alo_b'] = A_lo[0:128], A_lo[128:256]
    for x, A in (('hi', A_hi), ('lo', A_lo)):
        Me, Mo = s * A[0::2], s * A[1::2]                             # [128, 256]
        out[f'L1{x}_w1r'] = hs([Me, -Mo])
        out[f'L1{x}_w2r'] = hs([Me, Mo])
        out[f'L1{x}_w1i'] = hs([Mo, Me])
        out[f'L1{x}_w2i'] = hs([-Mo, Me])
    out['Be1_lo'], out['Bo1_lo'] = A_lo[0::2], A_lo[1::2]
    out['Be1_hi'], out['Bo1_hi'] = A_hi[0::2], A_hi[1::2]
    # zero-top variants: lhsT base partitions are limited to {0,32,64}, so
    # the 4th 32-row band slot (base 96) runs as K=64 at base 64 with the
    # top half of the matrix zeroed.
    z32 = np.zeros((32, 128))
    for nm in ('Be3_lo', 'Bo3_lo', 'Be3_hi', 'Bo3_hi'):
        out[nm + 'Z'] = np.vstack([z32, out[nm]])
    return {k: np.ascontiguousarray(v, np.float16) for k, v in out.items()}


MAT_SHAPES = {
    'M3_lo': (64, 128),
    'L3_hi_R': (64, 256), 'L3_hi_I': (64, 256),
    'L3_q_R': (128, 256), 'L3_q_I': (128, 256),
    'Be3_lo': (32, 128), 'Bo3_lo': (32, 128),
    'Be3_hi': (32, 128), 'Bo3_hi': (32, 128),
    'Be3_loZ': (64, 128), 'Bo3_loZ': (64, 128),
    'Be3_hiZ': (64, 128), 'Bo3_hiZ': (64, 128),
    'M2_lo': (128, 256),
    'L2_hi_R': (128, 512), 'L2_hi_I': (128, 512),
    'L2_lo_R': (128, 512), 'L2_lo_I': (128, 512),
    'Be2_lo': (64, 256), 'Bo2_lo': (64, 256),
    'Be2_hi': (64, 256), 'Bo2_hi': (64, 256),
    'Alo_a': (128, 256), 'Alo_b': (128, 256),
    'L1hi_w1r': (128, 512), 'L1hi_w2r': (128, 512),
    'L1hi_w1i': (128, 512), 'L1hi_w2i': (128, 512),
    'L1lo_w1r': (128, 512), 'L1lo_w2r': (128, 512),
    'L1lo_w1i': (128, 512), 'L1lo_w2i': (128, 512),
    'Be1_lo': (128, 256), 'Bo1_lo': (128, 256),
    'Be1_hi': (128, 256), 'Bo1_hi': (128, 256),
}

BLOB_A0 = ['M3_lo', 'L3_hi_R', 'L3_hi_I', 'L3_q_R', 'L3_q_I']
BLOB_A0R = ['Be3_lo', 'Bo3_lo', 'Be3_hi', 'Bo3_hi',
            'Be3_loZ', 'Bo3_loZ', 'Be3_hiZ', 'Bo3_hiZ']
BLOB_A1 = ['M2_lo', 'L2_hi_R', 'L2_hi_I', 'L2_lo_R', 'L2_lo_I',
           'Be2_lo', 'Bo2_lo', 'Be2_hi', 'Bo2_hi']
BLOB_B = ['Alo_a', 'Alo_b',
          'L1hi_w1r', 'L1hi_w2r', 'L1hi_w1i', 'L1hi_w2i',
          'L1lo_w1r', 'L1lo_w2r', 'L1lo_w1i', 'L1lo_w2i',
          'Be1_lo', 'Bo1_lo', 'Be1_hi', 'Bo1_hi']
BLOB_A0_COLS = sum(MAT_SHAPES[n][1] for n in BLOB_A0)
BLOB_A0R_COLS = sum(MAT_SHAPES[n][1] for n in BLOB_A0R)
BLOB_A1_COLS = sum(MAT_SHAPES[n][1] for n in BLOB_A1)
BLOB_B_COLS = sum(MAT_SHAPES[n][1] for n in BLOB_B)


def pack_blobs(mats):
    def pack(names, cols):
        blob = np.zeros((128, cols), np.float16)
        c = 0
        for n in names:
            K, N = MAT_SHAPES[n]
            for r in range(128 // K):  # replicate K<128 mats across parts
                blob[r * K:(r + 1) * K, c:c + N] = mats[n]
            c += N
        return blob
    return (pack(BLOB_A0, BLOB_A0_COLS), pack(BLOB_A0R, BLOB_A0R_COLS),
            pack(BLOB_A1, BLOB_A1_COLS), pack(BLOB_B, BLOB_B_COLS))


# ---------------------------------------------------------------------------
# Bass kernel
# ---------------------------------------------------------------------------
def split_excess_waits(nc, max_waits=1):
    """walrus CTRL codegen allows only one sem wait per instruction; move
    excess waits onto NoOps inserted just before the offending instruction."""
    ctr = 0
    for fn in nc.m.functions:
        for bb in fn.blocks:
            new_list = []
            for inst in bb.instructions:
                si = inst.sync_info
                if si is not None and si.on_wait and len(si.on_wait) > max_waits:
                    waits = list(si.on_wait)
                    keep, extra = waits[:max_waits], waits[max_waits:]
                    for i in range(0, len(extra), max_waits):
                        nop = mybir.InstNoOp(
                            name=f"wait_split_{ctr}", ins=[], outs=[])
                        ctr += 1
                        nop.engine = inst.engine
                        nop.sync_info = mybir.SyncInfo(
                            on_wait=extra[i:i + max_waits], on_update=[])
                        nc.register_instruction(nop)
                        new_list.append(nop)
                    inst.sync_info = mybir.SyncInfo(
                        on_wait=keep,
                        on_update=list(si.on_update) if si.on_update else [])
                new_list.append(inst)
            bb.instructions[:] = new_list
    return ctr


def build_nc():
    nc = bass.Bass()
    yl_d = nc.dram_tensor("yl", [IMGS_PER_CORE, 64, 64], F16,
                          kind="ExternalInput")
    yh2_d = nc.dram_tensor("yh2", [IMGS_PER_CORE, 6, 32, 32, 2], F16,
                           kind="ExternalInput")
    yh1_d = nc.dram_tensor("yh1", [IMGS_PER_CORE, 6, 64, 64, 2], F16,
                           kind="ExternalInput")
    yh0_d = nc.dram_tensor("yh0", [IMGS_PER_CORE, 6, 128, 128, 2], F16,
                           kind="ExternalInput")
    out_d = nc.dram_tensor("out", [IMGS_PER_CORE, 256, 256], F16,
                           kind="ExternalOutput")
    matsA0_d = nc.dram_tensor("matsA0", [128, BLOB_A0_COLS], F16,
                              kind="ExternalInput")
    matsA0R_d = nc.dram_tensor("matsA0R", [128, BLOB_A0R_COLS], F16,
                               kind="ExternalInput")
    matsA1_d = nc.dram_tensor("matsA1", [128, BLOB_A1_COLS], F16,
                              kind="ExternalInput")
    matsB_d = nc.dram_tensor("matsB", [128, BLOB_B_COLS], F16,
                             kind="ExternalInput")

    with TileContext(nc) as tc:
        with tc.tile_pool(name="mats", bufs=1) as matpool, \
             tc.tile_pool(name="ins", bufs=1) as inpool, \
             tc.tile_pool(name="zs", bufs=1) as zpool, \
             tc.tile_pool(name="mid", bufs=3) as midpool, \
             tc.tile_pool(name="outp", bufs=4) as outpool:

            # --- matrix blobs: separate tiles so deps stay fine-grained;
            # the tiny L3-col blob lands first so img0 starts early ---
            blobA0_t = matpool.tile([128, BLOB_A0_COLS], F16, tag="blobA0")
            nc.scalar.dma_start(out=blobA0_t[:], in_=matsA0_d[:])
            blobA0R_t = matpool.tile([128, BLOB_A0R_COLS], F16,
                                     tag="blobA0R")
            nc.scalar.dma_start(out=blobA0R_t[:], in_=matsA0R_d[:])
            blobA1_t = matpool.tile([128, BLOB_A1_COLS], F16, tag="blobA1")
            nc.scalar.dma_start(out=blobA1_t[:], in_=matsA1_d[:])
            blobB_t = matpool.tile([128, BLOB_B_COLS], F16, tag="blobB")
            nc.scalar.dma_start(out=blobB_t[:], in_=matsB_d[:])
            mats = {}
            mat_loc = {}
            for blob_t, names in ((blobA0_t, BLOB_A0), (blobA0R_t, BLOB_A0R),
                                  (blobA1_t, BLOB_A1), (blobB_t, BLOB_B)):
                c = 0
                for n in names:
                    K, N = MAT_SHAPES[n]
                    mats[n] = blob_t[0:K, c:c + N]
                    mat_loc[n] = (blob_t, c)
                    c += N

            def mat_at(name, poff):
                blob, c = mat_loc[name]
                K, N = MAT_SHAPES[name]
                return blob[poff:poff + K, c:c + N]

            # --- batched input sweeps ---
            # z3all[h, (i w)] <- yl[i, h, w]
            z3all = inpool.tile([64, 16 * 64], F16, tag="z3all")
            nc.gpsimd.dma_start(
                out=z3all.rearrange("h (i x) -> h i x", i=16),
                in_=yl_d.rearrange("i h x -> h i x"))
            # per-orientation all-image sweeps (DMA APs max 3 dims)
            def band_sweep(tile_ap, p0, p1, src5, i=16):
                nc.gpsimd.dma_start(
                    out=tile_ap[p0:p1, :].rearrange("h (i x) -> h i x", i=i),
                    in_=src5.rearrange("i h w r -> h i (w r)"))
            lh3all = inpool.tile([64, 16 * 64], F16, tag="lh3all")
            band_sweep(lh3all, 0, 32, yh2_d[:, 0])
            band_sweep(lh3all, 32, 64, yh2_d[:, 5])
            # q3all: parts 0:64 = orient pair (2,3), 64:128 = (1,4)
            q3all = inpool.tile([128, 16 * 64], F16, tag="q3all")
            band_sweep(q3all, 0, 32, yh2_d[:, 2])
            band_sweep(q3all, 32, 64, yh2_d[:, 3])
            band_sweep(q3all, 64, 96, yh2_d[:, 1])
            band_sweep(q3all, 96, 128, yh2_d[:, 4])
            # yh1 band pair tiles, one DMA per orientation
            lh2all = inpool.tile([128, 16 * 128], F16, tag="lh2all")
            band_sweep(lh2all, 0, 64, yh1_d[:, 0])
            band_sweep(lh2all, 64, 128, yh1_d[:, 5])
            hl2all = inpool.tile([128, 16 * 128], F16, tag="hl2all")
            band_sweep(hl2all, 0, 64, yh1_d[:, 2])
            band_sweep(hl2all, 64, 128, yh1_d[:, 3])
            hh2all = inpool.tile([128, 16 * 128], F16, tag="hh2all")
            band_sweep(hh2all, 0, 64, yh1_d[:, 1])
            band_sweep(hh2all, 64, 128, yh1_d[:, 4])
            # yh0: 4 groups of 4 imgs on the gpsimd queue (after yh1),
            # separate tiles so L1 deps stay per-group
            yh0g = {}
            for g in range(4):
                t = inpool.tile([128, 4 * 1536], F16, tag=f"yh0g{g}",
                                name=f"yh0g{g}")
                nc.gpsimd.dma_start(
                    out=t.rearrange("h (g x) -> h g x", g=24),
                    in_=yh0_d[4 * g:4 * g + 4].rearrange(
                        "i o h w r -> h (i o) (w r)"))
                yh0g[g] = t

            z2p = {p: zpool.tile([128, 256], F16, tag=f"z2p_{p}",
                                 name=f"z2p_{p}")
                   for p in range(IMGS_PER_CORE // 2)}
            z1p = {p: zpool.tile([128, 1024], F16, tag=f"z1p_{p}",
                                 name=f"z1p_{p}")
                   for p in range(IMGS_PER_CORE // 2)}

            def z2s_ap(img):
                return z2p[img // 2][:, (img % 2) * 128:(img % 2) * 128 + 128]

            def z1s_ap(img):
                return z1p[img // 2][:, (img % 2) * 512:(img % 2) * 512 + 512]

            def mm(out_ap, lhsT, rhs_name, start, stop, poff=0):
                rhs = mats[rhs_name] if poff == 0 else mat_at(rhs_name, poff)
                nc.tensor.matmul(out_ap, lhsT, rhs, start=start, stop=stop)

            # ===========================================================
            # Phase L3: quad-packed col stages; emission is software-
            # pipelined (col of quad g+1 precedes rows of quad g) so the
            # in-order PE queue never heads-of-line-blocks on copies
            # ===========================================================
            with tc.tile_pool(name="ps3c", bufs=2, space="PSUM") as ps3cpool,\
                 tc.tile_pool(name="ps3r", bufs=2, space="PSUM") as ps3rpool:
                l3t = {}

                def l3_col(g):
                    c0 = 4 * g * 64
                    p3 = ps3cpool.tile([128, 1024], F32, tag="p3",
                                       name=f"p3_{g}")
                    mm(p3[:, 0:128], z3all[:, c0:c0 + 128], 'M3_lo',
                       True, True)
                    mm(p3[:, 128:256], z3all[:, c0 + 128:c0 + 256], 'M3_lo',
                       True, True)
                    lq = lh3all[:, c0:c0 + 256]
                    mm(p3[:, 256:512], lq[:, 0::2], 'L3_hi_R', True, False)
                    mm(p3[:, 256:512], lq[:, 1::2], 'L3_hi_I', False, True)
                    qq = q3all[:, c0:c0 + 256]
                    mm(p3[:, 512:768], qq[:, 0::2], 'L3_q_R', True, False)
                    mm(p3[:, 512:768], qq[:, 1::2], 'L3_q_I', False, True)
                    y1z_s = midpool.tile([128, 256], F16, tag="y1z3",
                                         name=f"y1z3_{g}")
                    nc.scalar.copy(y1z_s[:], p3[:, 0:256])
                    y1b_s = midpool.tile([128, 256], F16, tag="y1b3",
                                         name=f"y1b3_{g}")
                    nc.vector.tensor_copy(out=y1b_s[:], in_=p3[:, 256:512])
                    y2b_s = midpool.tile([128, 256], F16, tag="y2b3",
                                         name=f"y2b3_{g}")
                    nc.vector.tensor_copy(out=y2b_s[:], in_=p3[:, 512:768])
                    l3t[g] = (y1z_s, y1b_s, y2b_s)

                def l3_row(g):
                    y1z_s, y1b_s, y2b_s = l3t.pop(g)
                    prow = ps3rpool.tile([128, 512], F32, tag="p3r",
                                         name=f"p3r_{g}")
                    for i in range(4):
                        img = 4 * g + i
                        zp = prow[:, i * 128:(i + 1) * 128]
                        zoff = (i % 2) * 64
                        zcol = (i // 2) * 128
                        mm(zp, y1z_s[zoff:zoff + 64, zcol:zcol + 128],
                           'M3_lo', True, False, poff=zoff)
                        if i < 3:
                            boff, sfx, bk = i * 32, '', 32
                        else:
                            boff, sfx, bk = 64, 'Z', 64
                        mm(zp, y1b_s[boff:boff + bk, 0:128],
                           'Be3_lo' + sfx, False, False, poff=boff)
                        mm(zp, y1b_s[boff:boff + bk, 128:256],
                           'Bo3_lo' + sfx, False, False, poff=boff)
                        mm(zp, y2b_s[boff:boff + bk, 0:128],
                           'Be3_hi' + sfx, False, False, poff=boff)
                        mm(zp, y2b_s[boff:boff + bk, 128:256],
                           'Bo3_hi' + sfx, False, True, poff=boff)
                        if i % 2 == 0:
                            nc.scalar.copy(z2s_ap(img), zp)
                        else:
                            nc.vector.tensor_copy(out=z2s_ap(img), in_=zp)

                l3_col(0)
                for g in range(1, 4):
                    l3_col(g)
                    l3_row(g - 1)
                l3_row(3)

            # ===========================================================
            # Phase L2: pair-packed col stages, software-pipelined with
            # per-img row stages (lhsT partition offsets)
            # ===========================================================
            with tc.tile_pool(name="ps2c", bufs=3, space="PSUM") as ps2cpool,\
                 tc.tile_pool(name="ps2r", bufs=2, space="PSUM") as ps2rpool:
                l2t = {}

                def l2_col(p):
                    cc = p * 256
                    pA = ps2cpool.tile([128, 512], F32, tag="p2",
                                       name=f"p2A_{p}")
                    mm(pA[:, 0:256], z2p[p][:, 0:128], 'M2_lo', True, True)
                    mm(pA[:, 256:512], z2p[p][:, 128:256], 'M2_lo',
                       True, True)
                    pB = ps2cpool.tile([128, 512], F32, tag="p2",
                                       name=f"p2B_{p}")
                    lp = lh2all[:, cc:cc + 256]
                    mm(pB[:], lp[:, 0::2], 'L2_hi_R', True, False)
                    mm(pB[:], lp[:, 1::2], 'L2_hi_I', False, True)
                    pC = ps2cpool.tile([128, 512], F32, tag="p2",
                                       name=f"p2C_{p}")
                    hp = hl2all[:, cc:cc + 256]
                    hq = hh2all[:, cc:cc + 256]
                    mm(pC[:], hp[:, 0::2], 'L2_lo_R', True, False)
                    mm(pC[:], hp[:, 1::2], 'L2_lo_I', False, False)
                    mm(pC[:], hq[:, 0::2], 'L2_hi_R', False, False)
                    mm(pC[:], hq[:, 1::2], 'L2_hi_I', False, True)
                    y1zT_s = midpool.tile([128, 512], F16, tag="y1zT2",
                                          name=f"y1zT2_{p}")
                    nc.scalar.copy(y1zT_s[:], pA[:])
                    b1_s = midpool.tile([128, 512], F16, tag="b1_2",
                                        name=f"b1_2_{p}")
                    nc.vector.tensor_copy(out=b1_s[:], in_=pB[:])
                    b2_s = midpool.tile([128, 512], F16, tag="b2_2",
                                        name=f"b2_2_{p}")
                    nc.vector.tensor_copy(out=b2_s[:], in_=pC[:])
                    l2t[p] = (y1zT_s, b1_s, b2_s)

                def l2_row(p):
                    y1zT_s, b1_s, b2_s = l2t.pop(p)
                    for i, img in enumerate((2 * p, 2 * p + 1)):
                        off = i * 64
                        abase = i * 256
                        p2r = ps2rpool.tile([128, 512], F32, tag="p2r",
                                            name=f"p2r_{img}")
                        for m in range(2):
                            zc = p2r[:, m * 256:(m + 1) * 256]
                            msl = slice(m * 128, (m + 1) * 128)
                            osl = slice(256 + m * 128, 256 + (m + 1) * 128)
                            mm(zc, y1zT_s[:, abase + m * 128:
                                          abase + (m + 1) * 128],
                               'M2_lo', True, False)
                            mm(zc, b1_s[off:off + 64, msl], 'Be2_lo',
                               False, False, poff=off)
                            mm(zc, b1_s[off:off + 64, osl], 'Bo2_lo',
                               False, False, poff=off)
                            mm(zc, b2_s[off:off + 64, msl], 'Be2_hi',
                               False, False, poff=off)
                            mm(zc, b2_s[off:off + 64, osl], 'Bo2_hi',
                               False, True, poff=off)
                            zdst = z1s_ap(img)
                            if m == 0:
                                nc.scalar.copy(zdst[:, 0:256], zc)
                            else:
                                nc.vector.tensor_copy(
                                    out=zdst[:, 256:512], in_=zc)

                l2_col(0)
                for p in range(1, 8):
                    l2_col(p)
                    l2_row(p - 1)
                l2_row(7)

            # ===========================================================
            # Phase L1: z1 + yh0 bands -> out, software-pipelined
            # ===========================================================
            with tc.tile_pool(name="ps1c", bufs=3, space="PSUM") as ps1cpool,\
                 tc.tile_pool(name="ps1r", bufs=2, space="PSUM") as ps1rpool:
                l1t = {}

                def l1_col(img):
                    yh0t = yh0g[img // 4]
                    ib = (img % 4) * 1536
                    o_t = {o: yh0t[:, ib + o * 256:ib + (o + 1) * 256]
                           for o in range(6)}
                    z1_s = z1s_ap(img)
                    # phase A: y1 = band + lowpass, merged in w-polyphase
                    # layout [E(h 256) | O(h 256)]  (partitions = w')
                    p1a = ps1cpool.tile([128, 512], F32, tag="p1",
                                        name=f"p1a_{img}")
                    y1_p = p1a[:]
                    mm(y1_p, o_t[0][:, 0::2], 'L1hi_w1r', True, False)
                    mm(y1_p, o_t[5][:, 0::2], 'L1hi_w2r', False, False)
                    mm(y1_p, o_t[0][:, 1::2], 'L1hi_w1i', False, False)
                    mm(y1_p, o_t[5][:, 1::2], 'L1hi_w2i', False, False)
                    mm(p1a[:, 0:256], z1_s[:, 0:256:2], 'Alo_a',
                       False, False)
                    mm(p1a[:, 0:256], z1_s[:, 256:512:2], 'Alo_b',
                       False, True)
                    mm(p1a[:, 256:512], z1_s[:, 1:256:2], 'Alo_a',
                       False, False)
                    mm(p1a[:, 256:512], z1_s[:, 257:512:2], 'Alo_b',
                       False, True)
                    y1_s = midpool.tile([128, 512], F16, tag="y1m",
                                        name=f"y1m_{img}")
                    nc.vector.tensor_copy(out=y1_s[:], in_=y1_p)

                    # phase B: y2b e|o [0:512)
                    p1b = ps1cpool.tile([128, 512], F32, tag="p1",
                                        name=f"p1b_{img}")
                    y2b_p = p1b[:]
                    mm(y2b_p, o_t[2][:, 0::2], 'L1lo_w1r', True, False)
                    mm(y2b_p, o_t[3][:, 0::2], 'L1lo_w2r', False, False)
                    mm(y2b_p, o_t[2][:, 1::2], 'L1lo_w1i', False, False)
                    mm(y2b_p, o_t[3][:, 1::2], 'L1lo_w2i', False, False)
                    mm(y2b_p, o_t[1][:, 0::2], 'L1hi_w1r', False, False)
                    mm(y2b_p, o_t[4][:, 0::2], 'L1hi_w2r', False, False)
                    mm(y2b_p, o_t[1][:, 1::2], 'L1hi_w1i', False, False)
                    mm(y2b_p, o_t[4][:, 1::2], 'L1hi_w2i', False, True)
                    y2b1_s = midpool.tile([128, 512], F16, tag="y2b1",
                                          name=f"y2b1_{img}")
                    nc.vector.tensor_copy(out=y2b1_s[:], in_=y2b_p)
                    l1t[img] = (y1_s, y2b1_s)

                def l1_row(img):
                    y1_s, y2b1_s = l1t.pop(img)
                    # row stage -> out [256, 256] in two h-chunks; single
                    # store DMA per image ([a p] x <- p [a x])
                    p1r = ps1rpool.tile([128, 512], F32, tag="p1r",
                                        name=f"p1r_{img}")
                    ot = outpool.tile([128, 512], F16, tag="ot",
                                      name=f"ot_{img}")
                    for m in range(2):
                        oc = p1r[:, m * 256:(m + 1) * 256]
                        msl = slice(m * 128, (m + 1) * 128)
                        osl = slice(256 + m * 128, 256 + (m + 1) * 128)
                        mm(oc, y1_s[:, msl], 'Be1_lo', True, False)
                        mm(oc, y1_s[:, osl], 'Bo1_lo', False, False)
                        mm(oc, y2b1_s[:, msl], 'Be1_hi', False, False)
                        mm(oc, y2b1_s[:, osl], 'Bo1_hi', False, True)
                        if m == 0:
                            nc.scalar.copy(ot[:, 0:256], oc)
                        else:
                            nc.vector.tensor_copy(out=ot[:, 256:512], in_=oc)
                    nc.sync.dma_start(
                        out=out_d[img].rearrange("(a p) x -> p a x", a=2),
                        in_=ot.rearrange("p (a x) -> p a x", a=2))

                l1_col(0)
                for img in range(1, IMGS_PER_CORE):
                    l1_col(img)
                    l1_row(img - 1)
                l1_row(IMGS_PER_CORE - 1)

    split_excess_waits(nc)
    return nc


# ---------------------------------------------------------------------------
# Entry point
# ---------------------------------------------------------------------------
_NC_CACHE = []
_LAST_RESULT = []  # last BassKernelResults (exec_time_ns when BASS_TRACE=1)


def _axon_reset():
    try:
        import ctypes
        lib = ctypes.CDLL('/opt/axon/libaxon_pjrt.so')
        lib.axon_reset.restype = ctypes.c_int64
        lib.axon_reset()
    except Exception:
        pass


def kernel(yl, yh0, yh1, yh2, g0o, g1o, g0a, g0b, g1a, g1b):
    yl = np.ascontiguousarray(np.asarray(yl, np.float16))
    yh0 = np.ascontiguousarray(np.asarray(yh0, np.float16))
    yh1 = np.ascontiguousarray(np.asarray(yh1, np.float16))
    yh2 = np.ascontiguousarray(np.asarray(yh2, np.float16))
    assert yl.shape == (8, 16, 64, 64)

    mats = build_matrices(g0o, g1o, g0a, g0b, g1a, g1b)
    blobA0, blobA0R, blobA1, blobB = pack_blobs(mats)
    if not _NC_CACHE:
        _NC_CACHE.append(build_nc())
    nc = _NC_CACHE[0]

    in_maps = []
    for core in range(N_CORES):
        m = {"yl": yl[core], "yh0": yh0[core],
             "yh1": yh1[core], "yh2": yh2[core],
             "matsA0": blobA0, "matsA0R": blobA0R,
             "matsA1": blobA1, "matsB": blobB}
        in_maps.append(m)

    try:
        res = run_bass_kernel_spmd(nc, in_maps, list(range(N_CORES)))
    except Exception as e:  # wedged exec unit: reset the axon device, retry
        if "UNAVAILABLE" not in str(e) and "unrecoverable" not in str(e):
            raise
        _axon_reset()
        res = run_bass_kernel_spmd(nc, in_maps, list(range(N_CORES)))
    _LAST_RESULT.clear()
    _LAST_RESULT.append(res)
    out = np.stack([res.results[i]["out"] for i in range(N_CORES)], axis=0)
    return np.ascontiguousarray(out.astype(np.float32))


# revision 29
# speedup vs baseline: 1.1346x; 1.1346x over previous
"""DTCWT 3-level inverse on 8 Trainium2 NeuronCores.

Every filtering stage is a banded matmul on the tensor engine in fp16
(PSUM accumulates fp32; ~7e-4 total rel err vs the 2e-2 gate).

All stages use "data as lhsT" mode: matmul(out, lhsT=data[K=h, M=w],
rhs=mat[K=h, N=h_out]) contracts over the partition dim of the data and
yields the filtered image TRANSPOSED ([w, h_out]); column and row stages
then alternate orientation naturally with zero explicit transposes.

The c2q band construction is folded into the matrices; at L1 the lowpass
path is additionally merged into the band polyphase layout ([E|O] w-planes)
so the final row stage is 4 accumulation passes instead of 6.

Schedule: phase-major (L3 x16, L2 x16, L1 x16) with double/triple-buffered
PSUM pools so neighbouring images' matmuls hide each other's copy latency.
DMA queue slots cost ~600ns regardless of size, so all loads are batched
into a handful of giant multi-dim DMAs (2 matrix blobs, 9 input sweeps,
1 store per image).

Sharding: pure data parallel over batch N (8 cores x 16 channels each).
"""
import sys

for _p in ('/opt/trn_rl_repo',):
    if _p not in sys.path:
        sys.path.append(_p)

import numpy as np
import concourse.bass as bass
import concourse.mybir as mybir
from concourse.tile import TileContext
from concourse.bass_utils import run_bass_kernel_spmd

SQRT_HALF = 0.7071067811865476
N_CORES = 8
IMGS_PER_CORE = 16
F32 = mybir.dt.float32
F16 = mybir.dt.float16


# ---------------------------------------------------------------------------
# Host-side matrix construction (numpy, float64)
# ---------------------------------------------------------------------------
def _conv_rows_valid(x, h):
    hr = h[::-1]
    taps = h.shape[0]
    n = x.shape[-2] - taps + 1
    out = hr[0] * x[..., 0:n, :]
    for k in range(1, taps):
        out = out + hr[k] * x[..., k:k + n, :]
    return out


def _pad_rows_symmetric(x, m):
    pad = [(0, 0)] * (x.ndim - 2) + [(m, m), (0, 0)]
    return np.pad(x, pad, mode='symmetric')


def _colfilter(x, h):
    return _conv_rows_valid(_pad_rows_symmetric(x, h.shape[0] // 2), h)


def _colifilt(x, ha, hb, highpass):
    m = ha.shape[0]
    m2 = m // 2
    r = x.shape[-2]
    xp = _pad_rows_symmetric(x, m2)
    xe = xp[..., 1:r + m - 2:2, :]
    xo = xp[..., 2:r + m - 1:2, :]
    xa, xb = (xe, xo) if highpass else (xo, xe)
    hao, hae = ha[0::2], ha[1::2]
    hbo, hbe = hb[0::2], hb[1::2]
    y0 = _conv_rows_valid(xb, hao)
    y1 = _conv_rows_valid(xa, hbo)
    y2 = _conv_rows_valid(xb, hae)
    y3 = _conv_rows_valid(xa, hbe)
    y = np.stack([y0, y1, y2, y3], axis=-2)
    return y.reshape(y.shape[:-3] + (2 * r, y.shape[-1]))


def _op_matrix(op, n):
    """M[h_in, h_out] with out[h_out, w] = sum_h M[h, h_out] x[h, w]."""
    return np.ascontiguousarray(op(np.eye(n, dtype=np.float64)).T)


def build_matrices(g0o, g1o, g0a, g0b, g1a, g1b):
    """All device matrices as {name: fp16 ndarray}."""
    g0o = np.asarray(g0o, np.float64)
    g1o = np.asarray(g1o, np.float64)
    g0a = np.asarray(g0a, np.float64)
    g0b = np.asarray(g0b, np.float64)
    g1a = np.asarray(g1a, np.float64)
    g1b = np.asarray(g1b, np.float64)
    s = SQRT_HALF
    hs, vs = np.hstack, np.vstack
    out = {}

    def upsample_level(R, tag):
        Mlo = _op_matrix(lambda x: _colifilt(x, g0b, g0a, False), R)  # [R, 2R]
        Mhi = _op_matrix(lambda x: _colifilt(x, g1b, g1a, True), R)
        Me_h, Mo_h = s * Mhi[0::2], s * Mhi[1::2]                     # [R/2, 2R]
        Me_l, Mo_l = s * Mlo[0::2], s * Mlo[1::2]
        out[f'M{tag}_lo'] = Mlo
        # pair-stacked [w1; w2] col rhs, e|o column-concatenated
        #   e: w1r*Me + w2r*Me + w1i*Mo - w2i*Mo
        #   o: -w1r*Mo + w2r*Mo + w1i*Me + w2i*Me
        out[f'L{tag}_hi_R'] = hs([vs([Me_h, Me_h]), vs([-Mo_h, Mo_h])])
        out[f'L{tag}_hi_I'] = hs([vs([Mo_h, -Mo_h]), vs([Me_h, Me_h])])
        out[f'L{tag}_lo_R'] = hs([vs([Me_l, Me_l]), vs([-Mo_l, Mo_l])])
        out[f'L{tag}_lo_I'] = hs([vs([Mo_l, -Mo_l]), vs([Me_l, Me_l])])
        # row stage (polyphase-column recombination)
        out[f'Be{tag}_lo'], out[f'Bo{tag}_lo'] = Mlo[0::2], Mlo[1::2]
        out[f'Be{tag}_hi'], out[f'Bo{tag}_hi'] = Mhi[0::2], Mhi[1::2]

    upsample_level(64, '3')
    upsample_level(128, '2')
    # L3 quad stacks: [hl pair (lo mats); hh pair (hi mats)], K=128
    out['L3_q_R'] = vs([out['L3_lo_R'], out['L3_hi_R']])
    out['L3_q_I'] = vs([out['L3_lo_I'], out['L3_hi_I']])
    del out['L3_lo_R'], out['L3_lo_I']  # only used inside the quad at L3

    # L1 (colfilter, size-preserving, n=256)
    A_lo = _op_matrix(lambda x: _colfilter(x, g0o), 256)              # [256, 256]
    A_hi = _op_matrix(lambda x: _colfilter(x, g1o), 256)
    out['Alo_a'], out['Alo_b'] = A_lo[0:128], A_lo[128:256]
    for x, A in (('hi', A_hi), ('lo', A_lo)):
        Me, Mo = s * A[0::2], s * A[1::2]                             # [128, 256]
        out[f'L1{x}_w1r'] = hs([Me, -Mo])
        out[f'L1{x}_w2r'] = hs([Me, Mo])
        out[f'L1{x}_w1i'] = hs([Mo, Me])
        out[f'L1{x}_w2i'] = hs([-Mo, Me])
    out['Be1_lo'], out['Bo1_lo'] = A_lo[0::2], A_lo[1::2]
    out['Be1_hi'], out['Bo1_hi'] = A_hi[0::2], A_hi[1::2]
    # block-diagonal row-stage variants: one wide matmul covers all the
    # images packed along the partition (K) dim of the quad/pair tiles.
    out['M3_bd2'] = np.kron(np.eye(2), out['M3_lo'])
    for nm in ('Be3_lo', 'Bo3_lo', 'Be3_hi', 'Bo3_hi'):
        out[nm + '_bd4'] = np.kron(np.eye(4), out[nm])
        del out[nm]
    for nm in ('Be2_lo', 'Bo2_lo', 'Be2_hi', 'Bo2_hi'):
        out[nm + '_bd2'] = np.kron(np.eye(2), out[nm])
        del out[nm]
    return {k: np.ascontiguousarray(v, np.float16) for k, v in out.items()}


MAT_SHAPES = {
    'M3_lo': (64, 128),
    'L3_hi_R': (64, 256), 'L3_hi_I': (64, 256),
    'L3_q_R': (128, 256), 'L3_q_I': (128, 256),
    'M3_bd2': (128, 256),
    'Be3_lo_bd4': (128, 512), 'Bo3_lo_bd4': (128, 512),
    'Be3_hi_bd4': (128, 512), 'Bo3_hi_bd4': (128, 512),
    'M2_lo': (128, 256),
    'L2_hi_R': (128, 512), 'L2_hi_I': (128, 512),
    'L2_lo_R': (128, 512), 'L2_lo_I': (128, 512),
    'Be2_lo_bd2': (128, 512), 'Bo2_lo_bd2': (128, 512),
    'Be2_hi_bd2': (128, 512), 'Bo2_hi_bd2': (128, 512),
    'Alo_a': (128, 256), 'Alo_b': (128, 256),
    'L1hi_w1r': (128, 512), 'L1hi_w2r': (128, 512),
    'L1hi_w1i': (128, 512), 'L1hi_w2i': (128, 512),
    'L1lo_w1r': (128, 512), 'L1lo_w2r': (128, 512),
    'L1lo_w1i': (128, 512), 'L1lo_w2i': (128, 512),
    'Be1_lo': (128, 256), 'Bo1_lo': (128, 256),
    'Be1_hi': (128, 256), 'Bo1_hi': (128, 256),
}

BLOB_A0 = ['M3_lo', 'L3_hi_R', 'L3_hi_I', 'L3_q_R', 'L3_q_I']
BLOB_A0R = ['M3_bd2', 'Be3_lo_bd4', 'Bo3_lo_bd4',
            'Be3_hi_bd4', 'Bo3_hi_bd4']
BLOB_A1 = ['M2_lo', 'L2_hi_R', 'L2_hi_I', 'L2_lo_R', 'L2_lo_I',
           'Be2_lo_bd2', 'Bo2_lo_bd2', 'Be2_hi_bd2', 'Bo2_hi_bd2']
BLOB_B = ['Alo_a', 'Alo_b',
          'L1hi_w1r', 'L1hi_w2r', 'L1hi_w1i', 'L1hi_w2i',
          'L1lo_w1r', 'L1lo_w2r', 'L1lo_w1i', 'L1lo_w2i',
          'Be1_lo', 'Bo1_lo', 'Be1_hi', 'Bo1_hi']
BLOB_A0_COLS = sum(MAT_SHAPES[n][1] for n in BLOB_A0)
BLOB_A0R_COLS = sum(MAT_SHAPES[n][1] for n in BLOB_A0R)
BLOB_A1_COLS = sum(MAT_SHAPES[n][1] for n in BLOB_A1)
BLOB_B_COLS = sum(MAT_SHAPES[n][1] for n in BLOB_B)


def pack_blobs(mats):
    def pack(names, cols):
        blob = np.zeros((128, cols), np.float16)
        c = 0
        for n in names:
            K, N = MAT_SHAPES[n]
            for r in range(128 // K):  # replicate K<128 mats across parts
                blob[r * K:(r + 1) * K, c:c + N] = mats[n]
            c += N
        return blob
    return (pack(BLOB_A0, BLOB_A0_COLS), pack(BLOB_A0R, BLOB_A0R_COLS),
            pack(BLOB_A1, BLOB_A1_COLS), pack(BLOB_B, BLOB_B_COLS))


# ---------------------------------------------------------------------------
# Bass kernel
# ---------------------------------------------------------------------------
def split_excess_waits(nc, max_waits=1):
    """walrus CTRL codegen allows only one sem wait per instruction; move
    excess waits onto NoOps inserted just before the offending instruction."""
    ctr = 0
    for fn in nc.m.functions:
        for bb in fn.blocks:
            new_list = []
            for inst in bb.instructions:
                si = inst.sync_info
                if si is not None and si.on_wait and len(si.on_wait) > max_waits:
                    waits = list(si.on_wait)
                    keep, extra = waits[:max_waits], waits[max_waits:]
                    for i in range(0, len(extra), max_waits):
                        nop = mybir.InstNoOp(
                            name=f"wait_split_{ctr}", ins=[], outs=[])
                        ctr += 1
                        nop.engine = inst.engine
                        nop.sync_info = mybir.SyncInfo(
                            on_wait=extra[i:i + max_waits], on_update=[])
                        nc.register_instruction(nop)
                        new_list.append(nop)
                    inst.sync_info = mybir.SyncInfo(
                        on_wait=keep,
                        on_update=list(si.on_update) if si.on_update else [])
                new_list.append(inst)
            bb.instructions[:] = new_list
    return ctr


def build_nc():
    nc = bass.Bass()
    yl_d = nc.dram_tensor("yl", [IMGS_PER_CORE, 64, 64], F16,
                          kind="ExternalInput")
    yh2_d = nc.dram_tensor("yh2", [IMGS_PER_CORE, 6, 32, 32, 2], F16,
                           kind="ExternalInput")
    yh1_d = nc.dram_tensor("yh1", [IMGS_PER_CORE, 6, 64, 64, 2], F16,
                           kind="ExternalInput")
    yh0_d = nc.dram_tensor("yh0", [IMGS_PER_CORE, 6, 128, 128, 2], F16,
                           kind="ExternalInput")
    out_d = nc.dram_tensor("out", [IMGS_PER_CORE, 256, 256], F16,
                           kind="ExternalOutput")
    matsA0_d = nc.dram_tensor("matsA0", [128, BLOB_A0_COLS], F16,
                              kind="ExternalInput")
    matsA0R_d = nc.dram_tensor("matsA0R", [128, BLOB_A0R_COLS], F16,
                               kind="ExternalInput")
    matsA1_d = nc.dram_tensor("matsA1", [128, BLOB_A1_COLS], F16,
                              kind="ExternalInput")
    matsB_d = nc.dram_tensor("matsB", [128, BLOB_B_COLS], F16,
                             kind="ExternalInput")

    with TileContext(nc) as tc:
        with tc.tile_pool(name="mats", bufs=1) as matpool, \
             tc.tile_pool(name="ins", bufs=1) as inpool, \
             tc.tile_pool(name="zs", bufs=1) as zpool, \
             tc.tile_pool(name="mid", bufs=3) as midpool, \
             tc.tile_pool(name="outp", bufs=4) as outpool:

            # --- matrix blobs: separate tiles so deps stay fine-grained;
            # the tiny L3-col blob lands first so img0 starts early ---
            blobA0_t = matpool.tile([128, BLOB_A0_COLS], F16, tag="blobA0")
            nc.scalar.dma_start(out=blobA0_t[:], in_=matsA0_d[:])
            blobA0R_t = matpool.tile([128, BLOB_A0R_COLS], F16,
                                     tag="blobA0R")
            nc.scalar.dma_start(out=blobA0R_t[:], in_=matsA0R_d[:])
            blobA1_t = matpool.tile([128, BLOB_A1_COLS], F16, tag="blobA1")
            nc.scalar.dma_start(out=blobA1_t[:], in_=matsA1_d[:])
            blobB_t = matpool.tile([128, BLOB_B_COLS], F16, tag="blobB")
            nc.scalar.dma_start(out=blobB_t[:], in_=matsB_d[:])
            mats = {}
            mat_loc = {}
            for blob_t, names in ((blobA0_t, BLOB_A0), (blobA0R_t, BLOB_A0R),
                                  (blobA1_t, BLOB_A1), (blobB_t, BLOB_B)):
                c = 0
                for n in names:
                    K, N = MAT_SHAPES[n]
                    mats[n] = blob_t[0:K, c:c + N]
                    mat_loc[n] = (blob_t, c)
                    c += N

            def mat_at(name, poff):
                blob, c = mat_loc[name]
                K, N = MAT_SHAPES[name]
                return blob[poff:poff + K, c:c + N]

            # --- batched input sweeps ---
            # z3all[h, (i w)] <- yl[i, h, w]
            z3all = inpool.tile([64, 16 * 64], F16, tag="z3all")
            nc.gpsimd.dma_start(
                out=z3all.rearrange("h (i x) -> h i x", i=16),
                in_=yl_d.rearrange("i h x -> h i x"))
            # per-orientation all-image sweeps (DMA APs max 3 dims)
            def band_sweep(tile_ap, p0, p1, src5, i=16):
                nc.gpsimd.dma_start(
                    out=tile_ap[p0:p1, :].rearrange("h (i x) -> h i x", i=i),
                    in_=src5.rearrange("i h w r -> h i (w r)"))
            lh3all = inpool.tile([64, 16 * 64], F16, tag="lh3all")
            band_sweep(lh3all, 0, 32, yh2_d[:, 0])
            band_sweep(lh3all, 32, 64, yh2_d[:, 5])
            # q3all: parts 0:64 = orient pair (2,3), 64:128 = (1,4)
            q3all = inpool.tile([128, 16 * 64], F16, tag="q3all")
            band_sweep(q3all, 0, 32, yh2_d[:, 2])
            band_sweep(q3all, 32, 64, yh2_d[:, 3])
            band_sweep(q3all, 64, 96, yh2_d[:, 1])
            band_sweep(q3all, 96, 128, yh2_d[:, 4])
            # yh1 band pair tiles, one DMA per orientation
            lh2all = inpool.tile([128, 16 * 128], F16, tag="lh2all")
            band_sweep(lh2all, 0, 64, yh1_d[:, 0])
            band_sweep(lh2all, 64, 128, yh1_d[:, 5])
            hl2all = inpool.tile([128, 16 * 128], F16, tag="hl2all")
            band_sweep(hl2all, 0, 64, yh1_d[:, 2])
            band_sweep(hl2all, 64, 128, yh1_d[:, 3])
            hh2all = inpool.tile([128, 16 * 128], F16, tag="hh2all")
            band_sweep(hh2all, 0, 64, yh1_d[:, 1])
            band_sweep(hh2all, 64, 128, yh1_d[:, 4])
            # yh0: 4 groups of 4 imgs on the gpsimd queue (after yh1),
            # separate tiles so L1 deps stay per-group
            yh0g = {}
            for g in range(4):
                t = inpool.tile([128, 4 * 1536], F16, tag=f"yh0g{g}",
                                name=f"yh0g{g}")
                nc.gpsimd.dma_start(
                    out=t.rearrange("h (g x) -> h g x", g=24),
                    in_=yh0_d[4 * g:4 * g + 4].rearrange(
                        "i o h w r -> h (i o) (w r)"))
                yh0g[g] = t

            z2p = {p: zpool.tile([128, 256], F16, tag=f"z2p_{p}",
                                 name=f"z2p_{p}")
                   for p in range(IMGS_PER_CORE // 2)}
            z1p = {p: zpool.tile([128, 1024], F16, tag=f"z1p_{p}",
                                 name=f"z1p_{p}")
                   for p in range(IMGS_PER_CORE // 2)}

            def z2s_ap(img):
                return z2p[img // 2][:, (img % 2) * 128:(img % 2) * 128 + 128]

            def z1s_ap(img):
                return z1p[img // 2][:, (img % 2) * 512:(img % 2) * 512 + 512]

            def mm(out_ap, lhsT, rhs_name, start, stop, poff=0):
                rhs = mats[rhs_name] if poff == 0 else mat_at(rhs_name, poff)
                nc.tensor.matmul(out_ap, lhsT, rhs, start=start, stop=stop)

            # ===========================================================
            # Phase L3: quad-packed col stages; emission is software-
            # pipelined (col of quad g+1 precedes rows of quad g) so the
            # in-order PE queue never heads-of-line-blocks on copies
            # ===========================================================
            with tc.tile_pool(name="ps3c", bufs=2, space="PSUM") as ps3cpool,\
                 tc.tile_pool(name="ps3r", bufs=2, space="PSUM") as ps3rpool:
                l3t = {}

                def l3_col(g):
                    c0 = 4 * g * 64
                    p3 = ps3cpool.tile([128, 1024], F32, tag="p3",
                                       name=f"p3_{g}")
                    mm(p3[:, 0:128], z3all[:, c0:c0 + 128], 'M3_lo',
                       True, True)
                    mm(p3[:, 128:256], z3all[:, c0 + 128:c0 + 256], 'M3_lo',
                       True, True)
                    lq = lh3all[:, c0:c0 + 256]
                    mm(p3[:, 256:512], lq[:, 0::2], 'L3_hi_R', True, False)
                    mm(p3[:, 256:512], lq[:, 1::2], 'L3_hi_I', False, True)
                    qq = q3all[:, c0:c0 + 256]
                    mm(p3[:, 512:768], qq[:, 0::2], 'L3_q_R', True, False)
                    mm(p3[:, 512:768], qq[:, 1::2], 'L3_q_I', False, True)
                    y1z_s = midpool.tile([128, 256], F16, tag="y1z3",
                                         name=f"y1z3_{g}")
                    nc.scalar.copy(y1z_s[:], p3[:, 0:256])
                    y1b_s = midpool.tile([128, 256], F16, tag="y1b3",
                                         name=f"y1b3_{g}")
                    nc.vector.tensor_copy(out=y1b_s[:], in_=p3[:, 256:512])
                    y2b_s = midpool.tile([128, 256], F16, tag="y2b3",
                                         name=f"y2b3_{g}")
                    nc.vector.tensor_copy(out=y2b_s[:], in_=p3[:, 512:768])
                    l3t[g] = (y1z_s, y1b_s, y2b_s)

                def l3_row(g):
                    y1z_s, y1b_s, y2b_s = l3t.pop(g)
                    prow = ps3rpool.tile([128, 512], F32, tag="p3r",
                                         name=f"p3r_{g}")
                    # imgs live at N cols 128*i via block-diagonal rhs;
                    # the full-width band mm starts the accumulation, the
                    # narrow lowpass mms accumulate into sub-ranges after
                    mm(prow[:], y1b_s[:, 0:128], 'Be3_lo_bd4', True, False)
                    mm(prow[:], y1b_s[:, 128:256], 'Bo3_lo_bd4',
                       False, False)
                    mm(prow[:], y2b_s[:, 0:128], 'Be3_hi_bd4', False, False)
                    mm(prow[:], y2b_s[:, 128:256], 'Bo3_hi_bd4',
                       False, False)
                    mm(prow[:, 0:256], y1z_s[:, 0:128], 'M3_bd2',
                       False, True)
                    mm(prow[:, 256:512], y1z_s[:, 128:256], 'M3_bd2',
                       False, True)
                    for i in range(4):
                        img = 4 * g + i
                        zp = prow[:, i * 128:(i + 1) * 128]
                        if i % 2 == 0:
                            nc.scalar.copy(z2s_ap(img), zp)
                        else:
                            nc.vector.tensor_copy(out=z2s_ap(img), in_=zp)

                l3_col(0)
                for g in range(1, 4):
                    l3_col(g)
                    l3_row(g - 1)
                l3_row(3)

            # ===========================================================
            # Phase L2: pair-packed col stages, software-pipelined with
            # per-img row stages (lhsT partition offsets)
            # ===========================================================
            with tc.tile_pool(name="ps2c", bufs=3, space="PSUM") as ps2cpool,\
                 tc.tile_pool(name="ps2r", bufs=2, space="PSUM") as ps2rpool:
                l2t = {}

                def l2_col(p):
                    cc = p * 256
                    pA = ps2cpool.tile([128, 512], F32, tag="p2",
                                       name=f"p2A_{p}")
                    mm(pA[:, 0:256], z2p[p][:, 0:128], 'M2_lo', True, True)
                    mm(pA[:, 256:512], z2p[p][:, 128:256], 'M2_lo',
                       True, True)
                    pB = ps2cpool.tile([128, 512], F32, tag="p2",
                                       name=f"p2B_{p}")
                    lp = lh2all[:, cc:cc + 256]
                    mm(pB[:], lp[:, 0::2], 'L2_hi_R', True, False)
                    mm(pB[:], lp[:, 1::2], 'L2_hi_I', False, True)
                    pC = ps2cpool.tile([128, 512], F32, tag="p2",
                                       name=f"p2C_{p}")
                    hp = hl2all[:, cc:cc + 256]
                    hq = hh2all[:, cc:cc + 256]
                    mm(pC[:], hp[:, 0::2], 'L2_lo_R', True, False)
                    mm(pC[:], hp[:, 1::2], 'L2_lo_I', False, False)
                    mm(pC[:], hq[:, 0::2], 'L2_hi_R', False, False)
                    mm(pC[:], hq[:, 1::2], 'L2_hi_I', False, True)
                    y1zT_s = midpool.tile([128, 512], F16, tag="y1zT2",
                                          name=f"y1zT2_{p}")
                    nc.scalar.copy(y1zT_s[:], pA[:])
                    b1_s = midpool.tile([128, 512], F16, tag="b1_2",
                                        name=f"b1_2_{p}")
                    nc.vector.tensor_copy(out=b1_s[:], in_=pB[:])
                    b2_s = midpool.tile([128, 512], F16, tag="b2_2",
                                        name=f"b2_2_{p}")
                    nc.vector.tensor_copy(out=b2_s[:], in_=pC[:])
                    l2t[p] = (y1zT_s, b1_s, b2_s)

                def l2_row(p):
                    y1zT_s, b1_s, b2_s = l2t.pop(p)
                    a, b = 2 * p, 2 * p + 1
                    for m in range(2):
                        # chunk m of both imgs: N = [a w_out 256 | b w_out
                        # 256], bands via block-diagonal rhs over the pair
                        p2r = ps2rpool.tile([128, 512], F32, tag="p2r",
                                            name=f"p2r_{p}_{m}")
                        msl = slice(m * 128, (m + 1) * 128)
                        osl = slice(256 + m * 128, 256 + (m + 1) * 128)
                        mm(p2r[:], b1_s[:, msl], 'Be2_lo_bd2', True, False)
                        mm(p2r[:], b1_s[:, osl], 'Bo2_lo_bd2', False, False)
                        mm(p2r[:], b2_s[:, msl], 'Be2_hi_bd2', False, False)
                        mm(p2r[:], b2_s[:, osl], 'Bo2_hi_bd2', False, False)
                        mm(p2r[:, 0:256], y1zT_s[:, m * 128:(m + 1) * 128],
                           'M2_lo', False, True)
                        mm(p2r[:, 256:512],
                           y1zT_s[:, 256 + m * 128:256 + (m + 1) * 128],
                           'M2_lo', False, True)
                        za, zb = z1s_ap(a), z1s_ap(b)
                        if m == 0:
                            nc.scalar.copy(za[:, 0:256], p2r[:, 0:256])
                            nc.vector.tensor_copy(out=zb[:, 0:256],
                                                  in_=p2r[:, 256:512])
                        else:
                            nc.scalar.copy(za[:, 256:512], p2r[:, 0:256])
                            nc.vector.tensor_copy(out=zb[:, 256:512],
                                                  in_=p2r[:, 256:512])

                l2_col(0)
                for p in range(1, 8):
                    l2_col(p)
                    l2_row(p - 1)
                l2_row(7)

            # ===========================================================
            # Phase L1: z1 + yh0 bands -> out, software-pipelined
            # ===========================================================
            with tc.tile_pool(name="ps1c", bufs=3, space="PSUM") as ps1cpool,\
                 tc.tile_pool(name="ps1r", bufs=2, space="PSUM") as ps1rpool:
                l1t = {}

                def l1_col(img):
                    yh0t = yh0g[img // 4]
                    ib = (img % 4) * 1536
                    o_t = {o: yh0t[:, ib + o * 256:ib + (o + 1) * 256]
                           for o in range(6)}
                    z1_s = z1s_ap(img)
                    # phase A: y1 = band + lowpass, merged in w-polyphase
                    # layout [E(h 256) | O(h 256)]  (partitions = w')
                    p1a = ps1cpool.tile([128, 512], F32, tag="p1",
                                        name=f"p1a_{img}")
                    y1_p = p1a[:]
                    mm(y1_p, o_t[0][:, 0::2], 'L1hi_w1r', True, False)
                    mm(y1_p, o_t[5][:, 0::2], 'L1hi_w2r', False, False)
                    mm(y1_p, o_t[0][:, 1::2], 'L1hi_w1i', False, False)
                    mm(y1_p, o_t[5][:, 1::2], 'L1hi_w2i', False, False)
                    mm(p1a[:, 0:256], z1_s[:, 0:256:2], 'Alo_a',
                       False, False)
                    mm(p1a[:, 0:256], z1_s[:, 256:512:2], 'Alo_b',
                       False, True)
                    mm(p1a[:, 256:512], z1_s[:, 1:256:2], 'Alo_a',
                       False, False)
                    mm(p1a[:, 256:512], z1_s[:, 257:512:2], 'Alo_b',
                       False, True)
                    y1_s = midpool.tile([128, 512], F16, tag="y1m",
                                        name=f"y1m_{img}")
                    nc.vector.tensor_copy(out=y1_s[:], in_=y1_p)

                    # phase B: y2b e|o [0:512)
                    p1b = ps1cpool.tile([128, 512], F32, tag="p1",
                                        name=f"p1b_{img}")
                    y2b_p = p1b[:]
                    mm(y2b_p, o_t[2][:, 0::2], 'L1lo_w1r', True, False)
                    mm(y2b_p, o_t[3][:, 0::2], 'L1lo_w2r', False, False)
                    mm(y2b_p, o_t[2][:, 1::2], 'L1lo_w1i', False, False)
                    mm(y2b_p, o_t[3][:, 1::2], 'L1lo_w2i', False, False)
                    mm(y2b_p, o_t[1][:, 0::2], 'L1hi_w1r', False, False)
                    mm(y2b_p, o_t[4][:, 0::2], 'L1hi_w2r', False, False)
                    mm(y2b_p, o_t[1][:, 1::2], 'L1hi_w1i', False, False)
                    mm(y2b_p, o_t[4][:, 1::2], 'L1hi_w2i', False, True)
                    y2b1_s = midpool.tile([128, 512], F16, tag="y2b1",
                                          name=f"y2b1_{img}")
                    nc.vector.tensor_copy(out=y2b1_s[:], in_=y2b_p)
                    l1t[img] = (y1_s, y2b1_s)

                def l1_row(img):
                    y1_s, y2b1_s = l1t.pop(img)
                    # row stage -> out [256, 256] in two h-chunks; single
                    # store DMA per image ([a p] x <- p [a x])
                    p1r = ps1rpool.tile([128, 512], F32, tag="p1r",
                                        name=f"p1r_{img}")
                    ot = outpool.tile([128, 512], F16, tag="ot",
                                      name=f"ot_{img}")
                    for m in range(2):
                        oc = p1r[:, m * 256:(m + 1) * 256]
                        msl = slice(m * 128, (m + 1) * 128)
                        osl = slice(256 + m * 128, 256 + (m + 1) * 128)
                        mm(oc, y1_s[:, msl], 'Be1_lo', True, False)
                        mm(oc, y1_s[:, osl], 'Bo1_lo', False, False)
                        mm(oc, y2b1_s[:, msl], 'Be1_hi', False, False)
                        mm(oc, y2b1_s[:, osl], 'Bo1_hi', False, True)
                        if m == 0:
                            nc.scalar.copy(ot[:, 0:256], oc)
                        else:
                            nc.vector.tensor_copy(out=ot[:, 256:512], in_=oc)
                    nc.sync.dma_start(
                        out=out_d[img].rearrange("(a p) x -> p a x", a=2),
                        in_=ot.rearrange("p (a x) -> p a x", a=2))

                l1_col(0)
                for img in range(1, IMGS_PER_CORE):
                    l1_col(img)
                    l1_row(img - 1)
                l1_row(IMGS_PER_CORE - 1)

    split_excess_waits(nc)
    return nc


# ---------------------------------------------------------------------------
# Entry point
# ---------------------------------------------------------------------------
_NC_CACHE = []
_LAST_RESULT = []  # last BassKernelResults (exec_time_ns when BASS_TRACE=1)


def _axon_reset():
    try:
        import ctypes
        lib = ctypes.CDLL('/opt/axon/libaxon_pjrt.so')
        lib.axon_reset.restype = ctypes.c_int64
        lib.axon_reset()
    except Exception:
        pass


def kernel(yl, yh0, yh1, yh2, g0o, g1o, g0a, g0b, g1a, g1b):
    yl = np.ascontiguousarray(np.asarray(yl, np.float16))
    yh0 = np.ascontiguousarray(np.asarray(yh0, np.float16))
    yh1 = np.ascontiguousarray(np.asarray(yh1, np.float16))
    yh2 = np.ascontiguousarray(np.asarray(yh2, np.float16))
    assert yl.shape == (8, 16, 64, 64)

    mats = build_matrices(g0o, g1o, g0a, g0b, g1a, g1b)
    blobA0, blobA0R, blobA1, blobB = pack_blobs(mats)
    if not _NC_CACHE:
        _NC_CACHE.append(build_nc())
    nc = _NC_CACHE[0]

    in_maps = []
    for core in range(N_CORES):
        m = {"yl": yl[core], "yh0": yh0[core],
             "yh1": yh1[core], "yh2": yh2[core],
             "matsA0": blobA0, "matsA0R": blobA0R,
             "matsA1": blobA1, "matsB": blobB}
        in_maps.append(m)

    try:
        res = run_bass_kernel_spmd(nc, in_maps, list(range(N_CORES)))
    except Exception as e:  # wedged exec unit: reset the axon device, retry
        if "UNAVAILABLE" not in str(e) and "unrecoverable" not in str(e):
            raise
        _axon_reset()
        res = run_bass_kernel_spmd(nc, in_maps, list(range(N_CORES)))
    _LAST_RESULT.clear()
    _LAST_RESULT.append(res)
    out = np.stack([res.results[i]["out"] for i in range(N_CORES)], axis=0)
    return np.ascontiguousarray(out.astype(np.float32))
